# revision 27
# baseline (speedup 1.0000x reference)
"""GCN joint-representation edge MLP on 8 TRN2 NeuronCores (Bass/Tile).

reference:
    node_rep = z[edge_index[0]] * z[edge_index[1]]          # [E, 64]
    joint    = concat([node_rep, edge_attr], -1)            # [E, 832]
    h        = relu(joint @ W1 + b1)                        # [E, 128]
    out      = softmax(h @ W2 + b2, -1)                     # [E, 5]

Sharding: pure data-parallel over edges, 8 cores x 25088 edges (E padded
200000 -> 200704, 0.35% pad).  Each core streams its edge slice as
24 DMA blocks of 1024 edges plus one final 512-edge block (the short
tail block both avoids padding bytes and halves the end-of-kernel
pipeline drain).

The kernel is memory-bound (target_regime=memory); two things dominate:
the stream size and the per-DMA fixed cost (~0.6us of serialized HWDGE
descriptor generation per dma_start).  Both are attacked directly:
  - edge_attr and the endpoint z-rows are cast to fp8 e4m3 (values
    ~N(0,1), well inside +-240).  attr is laid out for DoubleRow
    matmuls: 3 slices of 256-deep contraction at 2 MACs/cell/cycle.
    W1's attr rows are scaled x16 before the fp8 cast so ~N(0, 0.02)
    weights leave the subnormal floor; the scale is compensated exactly
    in W2 (relu is positively homogeneous and x16 is a power of two, so
    the transform is numerically free).
  - endpoint z-rows are resolved to dense per-edge streams host-side
    (device-side gather primitives are unusable in this runtime; the
    dense stream carries the same traffic an on-device gather would).
  - DMA count is minimized: attr moves in 768KB blocks, the z-stream in
    5-block batches, probs out in 10-c-block batches, and the tiny
    constants ride the gpsimd SWDGE ring so they never occupy the HWDGE
    rings at all.

Device pipeline per DMA block (L = 1024 or 512 edges):
  - node_rep = zz[zs]*zz[zd] (DVE, fp8 in, bf16 out)        [64, L]
  - per 512-edge half: 3 DoubleRow-fp8 + 1 bf16 accumulating matmuls
    -> hT [128, 512]; ScalarE relu(+16*b1) -> bf16
  - layer 2 in edge-major orientation: one K=1 bias matmul seeds b2 for
    the whole block, then per 128-edge chunk lhsT=hT[:,chunk] rhs=W2/16
    accumulates -> logits [128, ncb, 4, 5] (partition = edge in chunk)
  - softmax once per block at 128-lane width: ScalarE exp, DVE reduce
    over the 5 classes, fast reciprocal, one broadcast multiply
  - probs collect in a per-group tile, DMA'd per group; the host undoes
    the tiling.
"""
import numpy as np

import concourse.bass as bass
import concourse.bacc as bacc
import concourse.tile as tile
from concourse import mybir
from concourse.bass_utils import run_bass_kernel_spmd

F32 = mybir.dt.float32
BF16 = mybir.dt.bfloat16
F8 = mybir.dt.float8e4

N_CORES = 8
E_FULL = 200000
E_PAD = 200704              # 8 * 25088
E_CORE = E_PAD // N_CORES   # 25088 = 24*1024 + 512
SIZES = [1024] * 24 + [512]  # per-core DMA blocks
BOFF = [0]
for _L in SIZES:
    BOFF.append(BOFF[-1] + _L)
GRP = 5                     # DMA blocks per zz/out group
CB = 512                    # compute block (matmul N)
CH = CB // 128              # 4 edge chunks per compute block for layer 2
ZD = 64
AD = 768
NDS = AD // 256             # 3 DoubleRow slices (256 features each)
HID = 128
NCLS = 5
SLOT = CH * NCLS            # 20 floats per 512-edge c-block slot
WSCALE = 16.0               # power-of-two W1 prescale for fp8


def build_nc(nbd=len(SIZES), reps=1):
    """Per-core Bass program (same NEFF on all 8 cores).  `reps` wraps the
    block loop with a For_i for timing runs.  nbd must be a multiple of GRP."""
    assert nbd % GRP == 0
    ng = nbd // GRP
    sizes = SIZES[:nbd]
    ecore = sum(sizes)
    nslots = ecore // CB
    nc = bacc.Bacc("TRN2", target_bir_lowering=False, debug=False)

    attr8 = nc.declare_dram_parameter(
        "attr8", [128, NDS * 2 * ecore], F8, isOutput=False)
    zzT = nc.declare_dram_parameter("zzT", [ZD, 2 * ecore], F8, isOutput=False)
    w1a = nc.declare_dram_parameter("w1a", [ZD, HID], BF16, isOutput=False)
    w1d = nc.declare_dram_parameter(
        "w1d", [128, NDS * 2 * HID], F8, isOutput=False)
    w2 = nc.declare_dram_parameter("w2", [HID, NCLS], BF16, isOutput=False)
    b1 = nc.declare_dram_parameter("b1", [HID, 1], F32, isOutput=False)
    b2r = nc.declare_dram_parameter("b2r", [1, 2 * SLOT], BF16, isOutput=False)
    outp = nc.declare_dram_parameter(
        "outp", [128, nslots * SLOT], F32, isOutput=True)

    with tile.TileContext(nc) as tc:
        with (
            tc.tile_pool(name="const", bufs=1) as constp,
            tc.tile_pool(name="attrp", bufs=6) as attrp,
            tc.tile_pool(name="attrp2", bufs=1) as attrp2,
            tc.tile_pool(name="zp", bufs=2) as zp,
            tc.tile_pool(name="nrp", bufs=3) as nrp,
            tc.tile_pool(name="nrp2", bufs=1) as nrp2,
            tc.tile_pool(name="htp", bufs=4) as htp,
            tc.tile_pool(name="exp_", bufs=3) as expp,
            tc.tile_pool(name="outp_", bufs=3) as outpool,
            tc.tile_pool(name="ps_ht", bufs=5, space="PSUM") as ps_ht,
            tc.tile_pool(name="ps_lg", bufs=3, space="PSUM") as ps_lg,
        ):
            # ---- constants (SWDGE ring; keeps HWDGE free for the streams) ----
            w1a_t = constp.tile([ZD, HID], BF16)
            nc.gpsimd.dma_start(out=w1a_t[:], in_=w1a[:, :])
            w1d_t = constp.tile([128, NDS, 2, HID], F8)
            nc.gpsimd.dma_start(out=w1d_t[:], in_=w1d[:, :])
            w2_t = constp.tile([HID, NCLS], BF16)
            nc.gpsimd.dma_start(out=w2_t[:], in_=w2[:, :])
            b1_t = constp.tile([HID, 1], F32)
            nc.gpsimd.dma_start(out=b1_t[:], in_=b1[:, :])
            b2r_t = constp.tile([1, 2 * SLOT], BF16)
            nc.gpsimd.dma_start(out=b2r_t[:], in_=b2r[:, :])
            ones1_t = constp.tile([1, 128], BF16)
            nc.vector.memset(ones1_t[:], 1.0)

            def block(b, zz_t, zz_e0, pr_t, slot0):
                """One DMA block of sizes[b] edges; zz_e0 is the group's
                first edge, slot0 the block's first slot in pr_t."""
                L = sizes[b]
                ncb = L // CB
                if L == 1024:
                    attr_t = attrp.tile([128, NDS, 2, L], F8, tag="attr")
                    nr_t = nrp.tile([ZD, L], BF16, tag="nr")
                else:
                    attr_t = attrp2.tile([128, NDS, 2, L], F8, tag="attr2")
                    nr_t = nrp2.tile([ZD, L], BF16, tag="nr2")
                ao = NDS * 2 * BOFF[b]
                nc.sync.dma_start(
                    out=attr_t[:], in_=attr8[:, ao:ao + NDS * 2 * L])
                zo = 2 * (BOFF[b] - zz_e0)
                nc.vector.tensor_mul(
                    nr_t[:], zz_t[:, zo:zo + L], zz_t[:, zo + L:zo + 2 * L])

                hts = []
                for ci in range(ncb):
                    e0 = ci * CB
                    ht_ps = ps_ht.tile([HID, CB], F32, tag="htps")
                    for s in range(NDS):
                        nc.tensor.matmul(
                            out=ht_ps[:], lhsT=w1d_t[:, s],
                            rhs=attr_t[:, s, :, e0:e0 + CB],
                            start=(s == 0), stop=False,
                            perf_mode=mybir.MatmulPerfMode.DoubleRow,
                        )
                    nc.tensor.matmul(
                        out=ht_ps[:], lhsT=w1a_t[:],
                        rhs=nr_t[:, e0:e0 + CB],
                        start=False, stop=True,
                    )
                    ht_s = htp.tile([HID, CB], BF16, tag="hts")
                    nc.scalar.activation(
                        out=ht_s[:], in_=ht_ps[:],
                        func=mybir.ActivationFunctionType.Relu,
                        bias=b1_t[:],
                    )
                    hts.append(ht_s)

                # layer 2 + softmax for the whole block at once (uniform
                # [128, 2, CH, NCLS] tiles; a 512-edge block uses half)
                lg_ps = ps_lg.tile([128, 2, CH, NCLS], F32, tag="lgps")
                nc.tensor.matmul(
                    out=lg_ps[:], lhsT=ones1_t[:], rhs=b2r_t[:],
                    start=True, stop=False,
                )
                for ci in range(ncb):
                    for c in range(CH):
                        nc.tensor.matmul(
                            out=lg_ps[:, ci, c, :],
                            lhsT=hts[ci][:, c * 128:(c + 1) * 128],
                            rhs=w2_t[:],
                            start=False,
                            stop=(ci == ncb - 1 and c == CH - 1),
                        )
                ex_t = expp.tile([128, 2, CH, NCLS], F32, tag="ex")
                nc.scalar.activation(
                    out=ex_t[:, 0:ncb], in_=lg_ps[:, 0:ncb],
                    func=mybir.ActivationFunctionType.Exp,
                )
                sm_t = expp.tile([128, 2, CH], F32, tag="sm")
                nc.vector.tensor_reduce(
                    out=sm_t[:, 0:ncb], in_=ex_t[:, 0:ncb],
                    axis=mybir.AxisListType.X, op=mybir.AluOpType.add,
                )
                rc_t = expp.tile([128, 2, CH], F32, tag="rc")
                nc.vector.reciprocal_approx_fast(
                    out=rc_t[:, 0:ncb], in_=sm_t[:, 0:ncb])
                nc.vector.tensor_mul(
                    pr_t[:, slot0:slot0 + ncb], ex_t[:, 0:ncb],
                    rc_t[:, 0:ncb, :, None].broadcast_to([128, ncb, CH, NCLS]),
                )

            def group(g):
                b0, b1_ = g * GRP, (g + 1) * GRP
                e0, e1 = BOFF[b0], BOFF[b1_]
                zz_t = zp.tile([ZD, 2 * GRP * 1024], F8, tag="zz")
                nc.scalar.dma_start(
                    out=zz_t[:, 0:2 * (e1 - e0)],
                    in_=zzT[:, 2 * e0:2 * e1])
                gslots = (e1 - e0) // CB
                pr_t = outpool.tile([128, 2 * GRP, CH, NCLS], F32, tag="pr")
                for b in range(b0, b1_):
                    block(b, zz_t, e0, pr_t, (BOFF[b] - e0) // CB)
                oo = (e0 // CB) * SLOT
                nc.scalar.dma_start(
                    out=outp[:, oo:oo + gslots * SLOT],
                    in_=pr_t[:, 0:gslots])

            if reps == 1:
                for g in range(ng):
                    group(g)
            else:
                with tc.For_i(0, reps, 1):
                    for g in range(ng):
                        group(g)

    nc.compile()
    return nc


def _shard_inputs(z, edge_index, edge_attr, W1, b1, W2, b2):
    import ml_dtypes
    f8 = ml_dtypes.float8_e4m3
    bf16 = ml_dtypes.bfloat16
    z = np.asarray(z, dtype=np.float32)
    ei = np.asarray(edge_index).astype(np.int64)
    attr = np.asarray(edge_attr, dtype=np.float32)
    W1 = np.asarray(W1, dtype=np.float32)
    b1 = np.asarray(b1, dtype=np.float32)
    W2 = np.asarray(W2, dtype=np.float32)
    b2 = np.asarray(b2, dtype=np.float32)

    src = np.zeros(E_PAD, dtype=np.int64)
    dst = np.zeros(E_PAD, dtype=np.int64)
    src[:E_FULL] = ei[0]
    dst[:E_FULL] = ei[1]

    z8 = z.astype(f8)
    zs_all = z8[src]                       # [E_PAD, 64]
    zd_all = z8[dst]
    a8 = np.zeros((E_PAD, AD), dtype=f8)
    a8[:E_FULL] = attr.astype(f8)

    W1s = W1 * WSCALE
    w1a = W1s[:ZD].astype(bf16)
    w1d = np.ascontiguousarray(
        W1s[ZD:].reshape(NDS, 2, 128, HID).transpose(2, 0, 1, 3)
    ).reshape(128, NDS * 2 * HID).astype(f8)
    w2b = (W2 / WSCALE).astype(bf16)
    b1c = (b1 * WSCALE).reshape(HID, 1)
    b2rep = np.tile(b2, 2 * CH).reshape(1, 2 * SLOT).astype(bf16)

    in_maps = []
    for c in range(N_CORES):
        ce = slice(c * E_CORE, (c + 1) * E_CORE)
        ca, czs, czd = a8[ce], zs_all[ce], zd_all[ce]
        aparts, zparts = [], []
        for b, L in enumerate(SIZES):
            blk = ca[BOFF[b]:BOFF[b] + L]
            # feature f = s*256 + i*128 + p -> [128, s, i, e]
            aparts.append(np.ascontiguousarray(
                blk.reshape(L, NDS, 2, 128).transpose(3, 1, 2, 0)
            ).reshape(128, NDS * 2 * L))
            zparts.append(np.concatenate(
                [czs[BOFF[b]:BOFF[b] + L].T, czd[BOFF[b]:BOFF[b] + L].T],
                axis=1))
        in_maps.append({
            "attr8": np.concatenate(aparts, axis=1),
            "zzT": np.ascontiguousarray(np.concatenate(zparts, axis=1)),
            "w1a": w1a,
            "w1d": w1d,
            "w2": w2b,
            "b1": b1c,
            "b2r": b2rep,
        })
    return in_maps


def _gather_out(res_list):
    """[128, nslots*SLOT] per core -> [sum(edges), NCLS]."""
    outs = []
    for r in res_list:
        o = np.asarray(r["outp"], dtype=np.float32)
        nslots = o.shape[1] // SLOT
        o = o.reshape(128, nslots, CH, NCLS).transpose(1, 2, 0, 3)
        outs.append(o.reshape(nslots * CB, NCLS))
    return np.concatenate(outs, axis=0)


def kernel(z, edge_index, edge_attr, W1, b1, W2, b2):
    in_maps = _shard_inputs(z, edge_index, edge_attr, W1, b1, W2, b2)
    nc = build_nc()
    res = run_bass_kernel_spmd(nc, in_maps, core_ids=list(range(N_CORES))).results
    return np.ascontiguousarray(_gather_out(res)[:E_FULL])


# revision 28
# speedup vs baseline: 1.0023x; 1.0023x over previous
"""GCN joint-representation edge MLP on 8 TRN2 NeuronCores (Bass/Tile).

reference:
    node_rep = z[edge_index[0]] * z[edge_index[1]]          # [E, 64]
    joint    = concat([node_rep, edge_attr], -1)            # [E, 832]
    h        = relu(joint @ W1 + b1)                        # [E, 128]
    out      = softmax(h @ W2 + b2, -1)                     # [E, 5]

Sharding: pure data-parallel over edges, 8 cores x 25088 edges (E padded
200000 -> 200704, 0.35% pad).  Each core streams its edge slice as
24 DMA blocks of 1024 edges plus one final 512-edge block (the short
tail block both avoids padding bytes and halves the end-of-kernel
pipeline drain).

The kernel is memory-bound (target_regime=memory); two things dominate:
the stream size and the per-DMA fixed cost (~0.6us of serialized HWDGE
descriptor generation per dma_start).  Both are attacked directly:
  - edge_attr and the endpoint z-rows are cast to fp8 e4m3 (values
    ~N(0,1), well inside +-240).  attr is laid out for DoubleRow
    matmuls: 3 slices of 256-deep contraction at 2 MACs/cell/cycle.
    W1's attr rows are scaled x16 before the fp8 cast so ~N(0, 0.02)
    weights leave the subnormal floor; the scale is compensated exactly
    in W2 (relu is positively homogeneous and x16 is a power of two, so
    the transform is numerically free).
  - endpoint z-rows are resolved to dense per-edge streams host-side
    (device-side gather primitives are unusable in this runtime; the
    dense stream carries the same traffic an on-device gather would).
  - DMA count is minimized: attr moves in 768KB blocks, the z-stream in
    5-block batches, probs out in 10-c-block batches, and the tiny
    constants ride the gpsimd SWDGE ring so they never occupy the HWDGE
    rings at all.

Device pipeline per DMA block (L = 1024 or 512 edges):
  - node_rep = zz[zs]*zz[zd] (DVE, fp8 in, bf16 out)        [64, L]
  - per 512-edge half: 3 DoubleRow-fp8 + 1 bf16 accumulating matmuls
    -> hT [128, 512]; ScalarE relu(+16*b1) -> bf16
  - layer 2 in edge-major orientation: one K=1 bias matmul seeds b2 for
    the whole block, then per 128-edge chunk lhsT=hT[:,chunk] rhs=W2/16
    accumulates -> logits [128, ncb, 4, 5] (partition = edge in chunk)
  - softmax once per block at 128-lane width: ScalarE exp, DVE reduce
    over the 5 classes, fast reciprocal, one broadcast multiply
  - probs collect in a per-group tile, DMA'd per group; the host undoes
    the tiling.
"""
import numpy as np

import concourse.bass as bass
import concourse.bacc as bacc
import concourse.tile as tile
from concourse import mybir
from concourse.bass_utils import run_bass_kernel_spmd

F32 = mybir.dt.float32
BF16 = mybir.dt.bfloat16
F8 = mybir.dt.float8e4

N_CORES = 8
E_FULL = 200000
E_PAD = 200704              # 8 * 25088
E_CORE = E_PAD // N_CORES   # 25088 = 24*1024 + 512
SIZES = [2048] * 12 + [512]  # per-core DMA blocks
BOFF = [0]
for _L in SIZES:
    BOFF.append(BOFF[-1] + _L)
GRP = 1                     # DMA blocks per zz/out group
NCBM = 4                    # max compute blocks per DMA block
CB = 512                    # compute block (matmul N)
CH = CB // 128              # 4 edge chunks per compute block for layer 2
ZD = 64
AD = 768
NDS = AD // 256             # 3 DoubleRow slices (256 features each)
HID = 128
NCLS = 5
SLOT = CH * NCLS            # 20 floats per 512-edge c-block slot
WSCALE = 16.0               # power-of-two W1 prescale for fp8


def build_nc(nbd=len(SIZES), reps=1):
    """Per-core Bass program (same NEFF on all 8 cores).  `reps` wraps the
    block loop with a For_i for timing runs.  nbd must be a multiple of GRP."""
    assert nbd % GRP == 0
    ng = nbd // GRP
    sizes = SIZES[:nbd]
    ecore = sum(sizes)
    nslots = ecore // CB
    nc = bacc.Bacc("TRN2", target_bir_lowering=False, debug=False)

    attr8 = nc.declare_dram_parameter(
        "attr8", [128, NDS * 2 * ecore], F8, isOutput=False)
    zzT = nc.declare_dram_parameter("zzT", [ZD, 2 * ecore], F8, isOutput=False)
    w1a = nc.declare_dram_parameter("w1a", [ZD, HID], BF16, isOutput=False)
    w1d = nc.declare_dram_parameter(
        "w1d", [128, NDS * 2 * HID], F8, isOutput=False)
    w2 = nc.declare_dram_parameter("w2", [HID, NCLS], BF16, isOutput=False)
    b1 = nc.declare_dram_parameter("b1", [HID, 1], F32, isOutput=False)
    b2r = nc.declare_dram_parameter("b2r", [1, NCBM * SLOT], BF16, isOutput=False)
    outp = nc.declare_dram_parameter(
        "outp", [128, nslots * SLOT], F32, isOutput=True)

    with tile.TileContext(nc) as tc:
        with (
            tc.tile_pool(name="const", bufs=1) as constp,
            tc.tile_pool(name="attrp", bufs=3) as attrp,
            tc.tile_pool(name="attrp2", bufs=1) as attrp2,
            tc.tile_pool(name="zp", bufs=2) as zp,
            tc.tile_pool(name="nrp", bufs=3) as nrp,
            tc.tile_pool(name="nrp2", bufs=1) as nrp2,
            tc.tile_pool(name="htp", bufs=4) as htp,
            tc.tile_pool(name="exp_", bufs=3) as expp,
            tc.tile_pool(name="outp_", bufs=3) as outpool,
            tc.tile_pool(name="ps_ht", bufs=5, space="PSUM") as ps_ht,
            tc.tile_pool(name="ps_lg", bufs=3, space="PSUM") as ps_lg,
        ):
            # ---- constants (SWDGE ring; keeps HWDGE free for the streams) ----
            w1a_t = constp.tile([ZD, HID], BF16)
            nc.gpsimd.dma_start(out=w1a_t[:], in_=w1a[:, :])
            w1d_t = constp.tile([128, NDS, 2, HID], F8)
            nc.gpsimd.dma_start(out=w1d_t[:], in_=w1d[:, :])
            w2_t = constp.tile([HID, NCLS], BF16)
            nc.gpsimd.dma_start(out=w2_t[:], in_=w2[:, :])
            b1_t = constp.tile([HID, 1], F32)
            nc.gpsimd.dma_start(out=b1_t[:], in_=b1[:, :])
            b2r_t = constp.tile([1, NCBM * SLOT], BF16)
            nc.gpsimd.dma_start(out=b2r_t[:], in_=b2r[:, :])
            ones1_t = constp.tile([1, 128], BF16)
            nc.vector.memset(ones1_t[:], 1.0)

            def block(b, zz_t, zz_e0, pr_t, slot0):
                """One DMA block of sizes[b] edges; zz_e0 is the group's
                first edge, slot0 the block's first slot in pr_t."""
                L = sizes[b]
                ncb = L // CB
                if L == 2048:
                    attr_t = attrp.tile([128, NDS, 2, L], F8, tag="attr")
                    nr_t = nrp.tile([ZD, L], BF16, tag="nr")
                else:
                    attr_t = attrp2.tile([128, NDS, 2, L], F8, tag="attr2")
                    nr_t = nrp2.tile([ZD, L], BF16, tag="nr2")
                ao = NDS * 2 * BOFF[b]
                nc.sync.dma_start(
                    out=attr_t[:], in_=attr8[:, ao:ao + NDS * 2 * L])
                zo = 2 * (BOFF[b] - zz_e0)
                nc.vector.tensor_mul(
                    nr_t[:], zz_t[:, zo:zo + L], zz_t[:, zo + L:zo + 2 * L])

                hts = []
                for ci in range(ncb):
                    e0 = ci * CB
                    ht_ps = ps_ht.tile([HID, CB], F32, tag="htps")
                    for s in range(NDS):
                        nc.tensor.matmul(
                            out=ht_ps[:], lhsT=w1d_t[:, s],
                            rhs=attr_t[:, s, :, e0:e0 + CB],
                            start=(s == 0), stop=False,
                            perf_mode=mybir.MatmulPerfMode.DoubleRow,
                        )
                    nc.tensor.matmul(
                        out=ht_ps[:], lhsT=w1a_t[:],
                        rhs=nr_t[:, e0:e0 + CB],
                        start=False, stop=True,
                    )
                    ht_s = htp.tile([HID, CB], BF16, tag="hts")
                    nc.scalar.activation(
                        out=ht_s[:], in_=ht_ps[:],
                        func=mybir.ActivationFunctionType.Relu,
                        bias=b1_t[:],
                    )
                    hts.append(ht_s)

                # layer 2 + softmax for the whole block at once (uniform
                # [128, 2, CH, NCLS] tiles; a 512-edge block uses half)
                lg_ps = ps_lg.tile([128, NCBM, CH, NCLS], F32, tag="lgps")
                nc.tensor.matmul(
                    out=lg_ps[:], lhsT=ones1_t[:], rhs=b2r_t[:],
                    start=True, stop=False,
                )
                for ci in range(ncb):
                    for c in range(CH):
                        nc.tensor.matmul(
                            out=lg_ps[:, ci, c, :],
                            lhsT=hts[ci][:, c * 128:(c + 1) * 128],
                            rhs=w2_t[:],
                            start=False,
                            stop=(ci == ncb - 1 and c == CH - 1),
                        )
                ex_t = expp.tile([128, NCBM, CH, NCLS], F32, tag="ex")
                nc.scalar.activation(
                    out=ex_t[:, 0:ncb], in_=lg_ps[:, 0:ncb],
                    func=mybir.ActivationFunctionType.Exp,
                )
                sm_t = expp.tile([128, NCBM, CH], F32, tag="sm")
                nc.vector.tensor_reduce(
                    out=sm_t[:, 0:ncb], in_=ex_t[:, 0:ncb],
                    axis=mybir.AxisListType.X, op=mybir.AluOpType.add,
                )
                rc_t = expp.tile([128, NCBM, CH], F32, tag="rc")
                nc.vector.reciprocal_approx_fast(
                    out=rc_t[:, 0:ncb], in_=sm_t[:, 0:ncb])
                nc.vector.tensor_mul(
                    pr_t[:, slot0:slot0 + ncb], ex_t[:, 0:ncb],
                    rc_t[:, 0:ncb, :, None].broadcast_to([128, ncb, CH, NCLS]),
                )

            def group(g):
                b0, b1_ = g * GRP, (g + 1) * GRP
                e0, e1 = BOFF[b0], BOFF[b1_]
                zz_t = zp.tile([ZD, 2 * GRP * 2048], F8, tag="zz")
                nc.scalar.dma_start(
                    out=zz_t[:, 0:2 * (e1 - e0)],
                    in_=zzT[:, 2 * e0:2 * e1])
                gslots = (e1 - e0) // CB
                pr_t = outpool.tile([128, NCBM * GRP, CH, NCLS], F32, tag="pr")
                for b in range(b0, b1_):
                    block(b, zz_t, e0, pr_t, (BOFF[b] - e0) // CB)
                oo = (e0 // CB) * SLOT
                nc.scalar.dma_start(
                    out=outp[:, oo:oo + gslots * SLOT],
                    in_=pr_t[:, 0:gslots])

            if reps == 1:
                for g in range(ng):
                    group(g)
            else:
                with tc.For_i(0, reps, 1):
                    for g in range(ng):
                        group(g)

    nc.compile()
    return nc


def _shard_inputs(z, edge_index, edge_attr, W1, b1, W2, b2):
    import ml_dtypes
    f8 = ml_dtypes.float8_e4m3
    bf16 = ml_dtypes.bfloat16
    z = np.asarray(z, dtype=np.float32)
    ei = np.asarray(edge_index).astype(np.int64)
    attr = np.asarray(edge_attr, dtype=np.float32)
    W1 = np.asarray(W1, dtype=np.float32)
    b1 = np.asarray(b1, dtype=np.float32)
    W2 = np.asarray(W2, dtype=np.float32)
    b2 = np.asarray(b2, dtype=np.float32)

    src = np.zeros(E_PAD, dtype=np.int64)
    dst = np.zeros(E_PAD, dtype=np.int64)
    src[:E_FULL] = ei[0]
    dst[:E_FULL] = ei[1]

    z8 = z.astype(f8)
    zs_all = z8[src]                       # [E_PAD, 64]
    zd_all = z8[dst]
    a8 = np.zeros((E_PAD, AD), dtype=f8)
    a8[:E_FULL] = attr.astype(f8)

    W1s = W1 * WSCALE
    w1a = W1s[:ZD].astype(bf16)
    w1d = np.ascontiguousarray(
        W1s[ZD:].reshape(NDS, 2, 128, HID).transpose(2, 0, 1, 3)
    ).reshape(128, NDS * 2 * HID).astype(f8)
    w2b = (W2 / WSCALE).astype(bf16)
    b1c = (b1 * WSCALE).reshape(HID, 1)
    b2rep = np.tile(b2, 4 * CH).reshape(1, 4 * SLOT).astype(bf16)

    in_maps = []
    for c in range(N_CORES):
        ce = slice(c * E_CORE, (c + 1) * E_CORE)
        ca, czs, czd = a8[ce], zs_all[ce], zd_all[ce]
        aparts, zparts = [], []
        for b, L in enumerate(SIZES):
            blk = ca[BOFF[b]:BOFF[b] + L]
            # feature f = s*256 + i*128 + p -> [128, s, i, e]
            aparts.append(np.ascontiguousarray(
                blk.reshape(L, NDS, 2, 128).transpose(3, 1, 2, 0)
            ).reshape(128, NDS * 2 * L))
            zparts.append(np.concatenate(
                [czs[BOFF[b]:BOFF[b] + L].T, czd[BOFF[b]:BOFF[b] + L].T],
                axis=1))
        in_maps.append({
            "attr8": np.concatenate(aparts, axis=1),
            "zzT": np.ascontiguousarray(np.concatenate(zparts, axis=1)),
            "w1a": w1a,
            "w1d": w1d,
            "w2": w2b,
            "b1": b1c,
            "b2r": b2rep,
        })
    return in_maps


def _gather_out(res_list):
    """[128, nslots*SLOT] per core -> [sum(edges), NCLS]."""
    outs = []
    for r in res_list:
        o = np.asarray(r["outp"], dtype=np.float32)
        nslots = o.shape[1] // SLOT
        o = o.reshape(128, nslots, CH, NCLS).transpose(1, 2, 0, 3)
        outs.append(o.reshape(nslots * CB, NCLS))
    return np.concatenate(outs, axis=0)


def kernel(z, edge_index, edge_attr, W1, b1, W2, b2):
    in_maps = _shard_inputs(z, edge_index, edge_attr, W1, b1, W2, b2)
    nc = build_nc()
    res = run_bass_kernel_spmd(nc, in_maps, core_ids=list(range(N_CORES))).results
    return np.ascontiguousarray(_gather_out(res)[:E_FULL])
